# revision 1
# baseline (speedup 1.0000x reference)
"""GraphSAGE (2-layer, mean-agg) Trainium2 Bass kernel, 8-core SPMD.

Design: shard dst nodes across 8 cores (6250 each). Edges partitioned by dst
owner, sorted by dst, grouped into 128-dst windows. Messages fetched with
gpsimd dma_gather (bf16 tables, single_packet=False); segment-sum done on the
PE via per-rank selection-matrix matmuls accumulated in PSUM per window
(scatter-add CCE has a duplicate-index race on HW, so no scatters are used).
Layer-2 aggregates p = h @ w2_l (40->128-col padded bf16) instead of h
(512-dim): p shards are AllGathered in two 3125-row slices so gather indices
fit int16.
"""
import numpy as np
import ml_dtypes

N = 50000
E = 800000
DIN, HID, OUT = 128, 512, 40
NCORES = 8
NLOC = N // NCORES          # 6250
P = 128
NWIN = (NLOC + P - 1) // P  # 49
NPAD = NWIN * P             # 6272
XSPLIT = 32768              # x table split for int16 gather indices
SLICE_LEN = 1568            # p-table allgather slice length (4 slices)
NSLICE = 4
L1_CHUNK = 2                # windows per L1 gather call group
L2_CHUNK = 4

bf16 = ml_dtypes.bfloat16


def _build_schedule(edge_index):
    """Per-core, per-layer edge orderings + the cross-core-common rank schedule."""
    src = np.asarray(edge_index[0], dtype=np.int64)
    dst = np.asarray(edge_index[1], dtype=np.int64)
    deg = np.bincount(dst, minlength=N).astype(np.float32)
    recip = 1.0 / np.maximum(deg, 1.0)

    per_core = []
    for c in range(NCORES):
        lo, hi = c * NLOC, (c + 1) * NLOC
        m = (dst >= lo) & (dst < hi)
        s, d = src[m], dst[m] - lo
        per_core.append((s, d))

    # bucket key per layer: L1 by src>=XSPLIT, L2 by (src % NLOC) // PSLICE
    def buckets(s):
        return [s >= XSPLIT, (s % NLOC) // SLICE_LEN]

    # counts[layer][core][win][bucket]
    counts = np.zeros((2, NCORES, NWIN, 4), np.int64)
    percore_lists = []  # [core][layer][win][bucket] -> (gidx array, drel array)
    for c in range(NCORES):
        s, d = per_core[c]
        w = d // P
        bk = buckets(s)
        layers = []
        for L in range(2):
            nb = 2 if L == 0 else NSLICE
            b = bk[L].astype(np.int64)
            order = np.lexsort((b, w))  # by window, then bucket
            ss, dd, ww, bb = s[order], d[order], w[order], b[order]
            wins = []
            for wi in range(NWIN):
                sel = ww == wi
                ssw, ddw, bbw = ss[sel], dd[sel], bb[sel]
                ents = []
                for bu in range(nb):
                    q = bbw == bu
                    sq, dq = ssw[q], ddw[q]
                    if L == 0:
                        gi = np.where(sq >= XSPLIT, sq - XSPLIT, sq)
                    else:
                        u = sq % NLOC
                        gi = (sq // NLOC) * SLICE_LEN + (u - (u // SLICE_LEN) * SLICE_LEN)
                    counts[L, c, wi, bu] = len(sq)
                    ents.append((gi.astype(np.int64), (dq - wi * P).astype(np.int64)))
                wins.append(ents)
            layers.append(wins)
        percore_lists.append(layers)

    # common rank schedule: ranks[L][win][bucket] = ceil(max_c count /128), >=0
    ranks = np.zeros((2, NWIN, 4), np.int64)
    for L in range(2):
        mx = counts[L].max(axis=0)  # [NWIN, 2]
        ranks[L] = (mx + P - 1) // P
        for wi in range(NWIN):  # ensure every window has >=1 rank total
            if ranks[L, wi].sum() == 0:
                ranks[L, wi, 0] = 1
    return per_core, percore_lists, ranks, recip


def _wrap_call(flat_idx):
    """int16 wrapped layout for one gather call: slot i -> [i%16, i//16]."""
    n = len(flat_idx)
    w = flat_idx.astype(np.int16).reshape(n // 16, 16).T.copy()
    return np.tile(w, (8, 1))  # [128, n/16]


def _pack_layer(layers_for_core, ranks, L, chunk):
    """Build idx [128, T16] int16, drel [128, R] bf16 and call/window metadata.
    Call layout per chunk: [bucket0: win a..b segs][bucket1: win a..b segs]."""
    idx_cols, drel_cols = [], []
    calls = []      # (bucket, rank_off, n_ranks)
    win_ranges = [] # per window: list of (rank_start, rank_end)
    rank_off = 0
    for c0 in range(0, NWIN, chunk):
        cw = range(c0, min(c0 + chunk, NWIN))
        for bu in range(2 if L == 0 else NSLICE):
            seg_ranks = int(sum(ranks[L, wi, bu] for wi in cw))
            if seg_ranks == 0:
                continue
            flat = np.zeros(seg_ranks * P, np.int64)
            drel = np.full(seg_ranks * P, -1, np.int64)
            off = 0
            for wi in cw:
                nr = int(ranks[L, wi, bu])
                if nr == 0:
                    continue
                gi, dq = layers_for_core[wi][bu]
                flat[off:off + len(gi)] = gi
                drel[off:off + len(dq)] = dq
                if len(win_ranges) <= wi - 0:
                    pass
                win_ranges.append((wi, rank_off + off // P, rank_off + off // P + nr))
                off += nr * P
            idx_cols.append(_wrap_call(flat))
            # drel slot i -> partition i%128, rank i//128
            drel_cols.append(drel.reshape(seg_ranks, P).T.astype(bf16))
            calls.append((bu, rank_off, seg_ranks))
            rank_off += seg_ranks
    idx_arr = np.concatenate(idx_cols, axis=1)
    drel_arr = np.concatenate(drel_cols, axis=1)
    # merge win_ranges into per-window lists
    wmap = [[] for _ in range(NWIN)]
    for wi, a, b in win_ranges:
        wmap[wi].append((a, b))
    return idx_arr, drel_arr, calls, wmap


def kernel(x, edge_index, w1_l, b1, w1_r, w2_l, b2, w2_r):
    import concourse.bacc as bacc
    import concourse.bass as bass
    import concourse.mybir as mybir
    import concourse.tile as tile
    from concourse.bass_utils import run_bass_kernel_spmd
    from concourse.library_config import mlp
    from concourse.masks import make_identity

    x = np.asarray(x, np.float32)
    per_core, percore_lists, ranks, recip = _build_schedule(np.asarray(edge_index))

    # ---- host-side packed arrays (same shapes on every core) ----
    core_arrays = []
    for c in range(NCORES):
        i1, d1, calls1, wmap1 = _pack_layer(percore_lists[c][0], ranks, 0, L1_CHUNK)
        i2, d2, calls2, wmap2 = _pack_layer(percore_lists[c][1], ranks, 1, L2_CHUNK)
        core_arrays.append((i1, d1, i2, d2))
    calls1, wmap1, calls2, wmap2 = calls1, wmap1, calls2, wmap2  # same all cores

    xlo = np.zeros((XSPLIT, DIN), bf16); xlo[:] = x[:XSPLIT].astype(bf16)
    xhi = np.zeros((N - XSPLIT, DIN), bf16); xhi[:] = x[XSPLIT:].astype(bf16)
    iota_np = np.tile(np.arange(P, dtype=np.float32), (P, 1)).astype(bf16)
    b2b_np = np.tile(np.asarray(b2, np.float32)[None, :], (P, 1))

    T16_1, R1 = core_arrays[0][0].shape[1], core_arrays[0][1].shape[1]
    T16_2, R2 = core_arrays[0][2].shape[1], core_arrays[0][3].shape[1]

    nc = bacc.Bacc("TRN2")
    dt = mybir.dt
    t_xlo = nc.declare_dram_parameter("xlo", [XSPLIT, DIN], dt.bfloat16, isOutput=False)
    t_xhi = nc.declare_dram_parameter("xhi", [N - XSPLIT, DIN], dt.bfloat16, isOutput=False)
    t_xoT = nc.declare_dram_parameter("xoT", [P, NPAD], dt.bfloat16, isOutput=False)
    t_i1 = nc.declare_dram_parameter("i1", [P, T16_1], dt.int16, isOutput=False)
    t_d1 = nc.declare_dram_parameter("d1", [P, R1], dt.bfloat16, isOutput=False)
    t_i2 = nc.declare_dram_parameter("i2", [P, T16_2], dt.int16, isOutput=False)
    t_d2 = nc.declare_dram_parameter("d2", [P, R2], dt.bfloat16, isOutput=False)
    t_w1l = nc.declare_dram_parameter("w1l", [DIN, HID], dt.bfloat16, isOutput=False)
    t_w1r = nc.declare_dram_parameter("w1r", [DIN, HID], dt.bfloat16, isOutput=False)
    t_w2l = nc.declare_dram_parameter("w2l", [P, HID // P, OUT], dt.bfloat16, isOutput=False)
    t_w2r = nc.declare_dram_parameter("w2r", [P, HID // P, OUT], dt.bfloat16, isOutput=False)
    t_b1 = nc.declare_dram_parameter("b1", [P, HID // P], dt.float32, isOutput=False)
    t_b2 = nc.declare_dram_parameter("b2b", [P, OUT], dt.float32, isOutput=False)
    t_rc = nc.declare_dram_parameter("rc", [P, NWIN], dt.float32, isOutput=False)
    t_iota = nc.declare_dram_parameter("iota", [P, P], dt.bfloat16, isOutput=False)
    t_iota4 = nc.declare_dram_parameter("iota4", [P, 4, P], dt.bfloat16, isOutput=False)
    t_out = nc.declare_dram_parameter("out", [NPAD, OUT], dt.float32, isOutput=True)

    pS = [nc.dram_tensor(f"p{s}", [SLICE_LEN, P], dt.bfloat16) for s in range(NSLICE)]
    pgS = [nc.dram_tensor(f"pg{s}", [NCORES * SLICE_LEN, P], dt.bfloat16, addr_space="Shared") for s in range(NSLICE)]

    AluOp = mybir.AluOpType
    AF = mybir.ActivationFunctionType

    with tile.TileContext(nc) as tc:
        with tc.tile_pool(name="const", bufs=1) as cpool, \
             tc.tile_pool(name="msg", bufs=2) as mpool, \
             tc.tile_pool(name="sm", bufs=3) as spool, \
             tc.tile_pool(name="work", bufs=3) as wpool, \
             tc.tile_pool(name="psumA", bufs=2, space="PSUM") as ppool, \
             tc.tile_pool(name="psumB", bufs=1, space="PSUM") as ppoolb:
            nc.gpsimd.load_library(mlp)
            ident = cpool.tile([P, P], dt.bfloat16)
            make_identity(nc, ident[:])
            iota_t = cpool.tile([P, P], dt.bfloat16)
            nc.sync.dma_start(iota_t[:], t_iota[:])
            iota4_t = cpool.tile([P, 4, P], dt.bfloat16)
            nc.sync.dma_start(iota4_t[:], t_iota4[:])
            i1_t = cpool.tile([P, T16_1], dt.int16)
            nc.sync.dma_start(i1_t[:], t_i1[:])
            d1_t = cpool.tile([P, R1], dt.bfloat16)
            nc.sync.dma_start(d1_t[:], t_d1[:])
            i2_t = cpool.tile([P, T16_2], dt.int16)
            nc.sync.dma_start(i2_t[:], t_i2[:])
            d2_t = cpool.tile([P, R2], dt.bfloat16)
            nc.sync.dma_start(d2_t[:], t_d2[:])
            xoT_t = cpool.tile([P, NPAD], dt.bfloat16)
            nc.sync.dma_start(xoT_t[:], t_xoT[:])
            w1l_t = cpool.tile([DIN, HID], dt.bfloat16)
            nc.sync.dma_start(w1l_t[:], t_w1l[:])
            w1r_t = cpool.tile([DIN, HID], dt.bfloat16)
            nc.sync.dma_start(w1r_t[:], t_w1r[:])
            w2l_t = cpool.tile([P, HID // P, OUT], dt.bfloat16)
            nc.sync.dma_start(w2l_t[:], t_w2l[:])
            w2r_t = cpool.tile([P, HID // P, OUT], dt.bfloat16)
            nc.sync.dma_start(w2r_t[:], t_w2r[:])
            b1_t = cpool.tile([P, HID // P], dt.float32)
            nc.sync.dma_start(b1_t[:], t_b1[:])
            b2_t = cpool.tile([P, OUT], dt.float32)
            nc.sync.dma_start(b2_t[:], t_b2[:])
            rc_t = cpool.tile([P, NWIN], dt.float32)
            nc.sync.dma_start(rc_t[:], t_rc[:])
            qbuf = cpool.tile([P, NWIN, OUT], dt.float32)

            # ---------- Layer 1 + stage B, chunked ----------
            call_i = 0
            cum16 = 0
            for c0 in range(0, NWIN, L1_CHUNK):
                cw = list(range(c0, min(c0 + L1_CHUNK, NWIN)))
                chunk_ranks = int(sum(ranks[0, wi, :].sum() for wi in cw))
                if chunk_ranks == 0:
                    continue
                msg = mpool.tile([P, chunk_ranks, DIN], dt.bfloat16, tag="msg1")
                base_rank = None
                # issue this chunk's gather calls
                local_off = 0
                while call_i < len(calls1):
                    bu, roff, nr = calls1[call_i]
                    # does this call belong to the current chunk?
                    if base_rank is None:
                        base_rank = roff
                    if roff - base_rank >= chunk_ranks:
                        break
                    n_idx = nr * P
                    tblap = t_xlo[:] if bu == 0 else t_xhi[:]
                    nc.gpsimd.dma_gather(
                        msg[:, roff - base_rank:roff - base_rank + nr, :],
                        tblap, i1_t[:, cum16:cum16 + n_idx // 16],
                        n_idx, n_idx, DIN, single_packet=False)
                    cum16 += n_idx // 16
                    local_off += nr
                    call_i += 1
                # per-window segmented reduction + stage B
                for wi in cw:
                    segs = [(a - base_rank, b - base_rank) for a, b in wmap1[wi]]
                    nseg = sum(b - a for a, b in segs)
                    pagg = ppool.tile([P, P], dt.float32, tag="pagg")
                    first = True
                    for a, b in segs:
                        r = a
                        while r < b:
                            kk = min(4, b - r)
                            S = spool.tile([P, 4, P], dt.bfloat16, tag="S1")
                            nc.vector.tensor_tensor(
                                out=S[:, :kk, :],
                                in0=d1_t[:, base_rank + r:base_rank + r + kk, None].to_broadcast([P, kk, P]),
                                in1=iota4_t[:, :kk, :], op=AluOp.is_equal)
                            for j in range(kk):
                                nc.tensor.matmul(pagg[:], lhsT=S[:, j, :], rhs=msg[:, r + j, :],
                                                 start=first, stop=(r + j == b - 1 and (a, b) == segs[-1]))
                                first = False
                            r += kk
                    am = wpool.tile([P, DIN], dt.bfloat16, tag="am")
                    nc.scalar.activation(am[:], pagg[:], AF.Copy, scale=rc_t[:, wi:wi + 1])
                    pamT = ppoolb.tile([P, P], dt.bfloat16, tag="pamT")
                    nc.tensor.transpose(out=pamT[:], in_=am[:], identity=ident[:])
                    amT = wpool.tile([P, P], dt.bfloat16, tag="amT")
                    nc.scalar.activation(amT[:], pamT[:], AF.Copy)
                    # h blocks + p/q
                    pq = ppool.tile([P, OUT], dt.float32, tag="pq")
                    qq = ppool.tile([P, OUT], dt.float32, tag="qq")
                    for bjj in range(HID // P):
                        ph = ppoolb.tile([P, P], dt.float32, tag="ph")
                        nc.tensor.matmul(ph[:], lhsT=w1l_t[:, bjj * P:(bjj + 1) * P], rhs=amT[:], start=True, stop=False)
                        nc.tensor.matmul(ph[:], lhsT=w1r_t[:, bjj * P:(bjj + 1) * P], rhs=xoT_t[:, wi * P:(wi + 1) * P], start=False, stop=True)
                        hT = wpool.tile([P, P], dt.bfloat16, tag="hT")
                        nc.scalar.activation(hT[:], ph[:], AF.Relu, bias=b1_t[:, bjj:bjj + 1])
                        nc.tensor.matmul(pq[:], lhsT=hT[:], rhs=w2l_t[:, bjj, :], start=(bjj == 0), stop=(bjj == 3))
                        nc.tensor.matmul(qq[:], lhsT=hT[:], rhs=w2r_t[:, bjj, :], start=(bjj == 0), stop=(bjj == 3))
                    nc.scalar.activation(qbuf[:, wi, :], qq[:], AF.Copy)
                    pt = wpool.tile([P, P], dt.bfloat16, tag="pt")
                    nc.vector.memset(pt[:], 0.0)
                    nc.scalar.activation(pt[:, :OUT], pq[:], AF.Copy)
                    r0, r1_ = wi * P, min((wi + 1) * P, NLOC)
                    for s in range(NSLICE):
                        a0, a1 = s * SLICE_LEN, min((s + 1) * SLICE_LEN, NLOC)
                        c0_, c1_ = max(r0, a0), min(r1_, a1)
                        if c0_ < c1_:
                            nc.sync.dma_start(pS[s][c0_ - a0:c1_ - a0, :], pt[c0_ - r0:c1_ - r0, :])

            # ---------- AllGather p slices ----------
            for s in range(NSLICE):
                nc.gpsimd.collective_compute(
                    "AllGather", AluOp.bypass, replica_groups=[list(range(NCORES))],
                    ins=[pS[s][:]], outs=[pgS[s][:]])

            # ---------- Layer 2 + output ----------
            call_i = 0
            cum16 = 0
            for c0 in range(0, NWIN, L2_CHUNK):
                cw = list(range(c0, min(c0 + L2_CHUNK, NWIN)))
                chunk_ranks = int(sum(ranks[1, wi, :].sum() for wi in cw))
                if chunk_ranks == 0:
                    continue
                msg = mpool.tile([P, chunk_ranks, P], dt.bfloat16, tag="msg2")
                base_rank = None
                while call_i < len(calls2):
                    bu, roff, nr = calls2[call_i]
                    if base_rank is None:
                        base_rank = roff
                    if roff - base_rank >= chunk_ranks:
                        break
                    n_idx = nr * P
                    tblap = pgS[bu][:]
                    nc.gpsimd.dma_gather(
                        msg[:, roff - base_rank:roff - base_rank + nr, :],
                        tblap, i2_t[:, cum16:cum16 + n_idx // 16],
                        n_idx, n_idx, P, single_packet=False)
                    cum16 += n_idx // 16
                    call_i += 1
                for wi in cw:
                    segs = [(a - base_rank, b - base_rank) for a, b in wmap2[wi]]
                    pagg = ppool.tile([P, P], dt.float32, tag="pagg")
                    first = True
                    for a, b in segs:
                        r = a
                        while r < b:
                            kk = min(4, b - r)
                            S = spool.tile([P, 4, P], dt.bfloat16, tag="S2")
                            nc.vector.tensor_tensor(
                                out=S[:, :kk, :],
                                in0=d2_t[:, base_rank + r:base_rank + r + kk, None].to_broadcast([P, kk, P]),
                                in1=iota4_t[:, :kk, :], op=AluOp.is_equal)
                            for j in range(kk):
                                nc.tensor.matmul(pagg[:], lhsT=S[:, j, :], rhs=msg[:, r + j, :],
                                                 start=first, stop=(r + j == b - 1 and (a, b) == segs[-1]))
                                first = False
                            r += kk
                    z = wpool.tile([P, OUT], dt.float32, tag="z")
                    nc.vector.tensor_tensor(out=z[:], in0=pagg[:, :OUT],
                                            in1=rc_t[:, wi:wi + 1].to_broadcast([P, OUT]),
                                            op=AluOp.mult)
                    nc.vector.tensor_tensor(out=z[:], in0=z[:], in1=qbuf[:, wi, :], op=AluOp.add)
                    nc.vector.tensor_tensor(out=z[:], in0=z[:], in1=b2_t[:], op=AluOp.add)
                    mneg = wpool.tile([P, 1], dt.float32, tag="mneg")
                    nc.vector.tensor_reduce(mneg[:], z[:], axis=mybir.AxisListType.X, op=AluOp.max, negate=True)
                    ez = wpool.tile([P, OUT], dt.float32, tag="ez")
                    nc.scalar.activation(ez[:], z[:], AF.Exp, bias=mneg[:])
                    ssum = wpool.tile([P, 1], dt.float32, tag="ssum")
                    nc.vector.tensor_reduce(ssum[:], ez[:], axis=mybir.AxisListType.X, op=AluOp.add)
                    lsum = wpool.tile([P, 1], dt.float32, tag="lsum")
                    nc.scalar.activation(lsum[:], ssum[:], AF.Ln)
                    nc.vector.tensor_tensor(out=lsum[:], in0=lsum[:], in1=mneg[:], op=AluOp.subtract)
                    ot = wpool.tile([P, OUT], dt.float32, tag="ot")
                    nc.vector.tensor_tensor(out=ot[:], in0=z[:], in1=lsum[:].to_broadcast([P, OUT]), op=AluOp.subtract)
                    nc.sync.dma_start(t_out[wi * P:(wi + 1) * P, :], ot[:])

    nc.compile()

    in_maps = []
    for c in range(NCORES):
        i1a, d1a, i2a, d2a = core_arrays[c]
        xoT = np.zeros((P, NPAD), bf16)
        xoT[:, :NLOC] = x[c * NLOC:(c + 1) * NLOC].T.astype(bf16)
        rcf = np.ones(NPAD, np.float32)
        rcf[:NLOC] = recip[c * NLOC:(c + 1) * NLOC]
        rcc = rcf.reshape(NWIN, P).T.copy()
        in_maps.append({
            "xlo": xlo, "xhi": xhi, "xoT": xoT,
            "i1": i1a, "d1": d1a, "i2": i2a, "d2": d2a,
            "w1l": np.asarray(w1_l).astype(bf16), "w1r": np.asarray(w1_r).astype(bf16),
            "w2l": np.ascontiguousarray(np.asarray(w2_l).astype(bf16).reshape(HID // P, P, OUT).transpose(1, 0, 2)), "w2r": np.ascontiguousarray(np.asarray(w2_r).astype(bf16).reshape(HID // P, P, OUT).transpose(1, 0, 2)),
            "b1": np.asarray(b1, np.float32).reshape(HID // P, P).T.copy(),
            "b2b": b2b_np, "rc": rcc,
            "iota": iota_np, "iota4": np.ascontiguousarray(np.broadcast_to(iota_np[:, None, :], (128, 4, 128))),
        })
    res = run_bass_kernel_spmd(nc, in_maps, list(range(NCORES)))
    out = np.concatenate([res.results[c]["out"][:NLOC] for c in range(NCORES)], axis=0)
    kernel.last_results = res
    kernel.last_nc = nc
    return out.astype(np.float32)



# revision 13
# speedup vs baseline: 2.0617x; 2.0617x over previous
"""GraphSAGE (2-layer, mean-agg) Trainium2 Bass kernel, 8-core SPMD.

Design v2 (vs the AllGather baseline):
- L1 dst-partitioned: each core owns 6250 dst nodes; x tables (xlo/xhi bf16)
  replicated in DRAM; per-edge messages fetched with gpsimd dma_gather;
  segment-sum on the PE via per-rank selection matmuls (S built on DVE with
  tensor_scalar is_equal against an iota table — one inst/rank, 4x_2p rate).
- Dense rank packing (no per-window padding): edges sorted by window are
  packed densely into 128-slot ranks; a rank may span up to 4 adjacent
  windows.  drel = slot + 128*(win&3) (mod-512 phase trick) lets one S hold
  up to 4 window blocks; per-core coverage differences are absorbed by the
  cross-core union of windows per rank (non-matching cores contribute zero).
- L2 src-partitioned: each core aggregates p = h@w2_l (40-dim) of its OWN
  nodes into partial sums over ALL 50176 dst slots, so the only collective
  is one small ReduceScatter(add) of [128, 392, 40] bf16 at the end —
  replacing 4 serial AllGathers (381us) with one 27us collective.
- Node layouts: L1 local node l = p*49 + k (partition p, window k) so qbuf /
  rc / epilogue all share one layout; L2 padded global node n = p2*392 + w2
  so the ReduceScatter shard (axis0 16-partition block) is exactly one
  core's nodes.
- Epilogue (recip-scale, +q, +b2, log_softmax) runs once, batched
  [128,49,40], avoiding per-window Exp/Ln activation-table thrash.
"""
import numpy as np
import ml_dtypes

N = 50000
E = 800000
DIN, HID, OUT = 128, 512, 40
NCORES = 8
NLOC = N // NCORES            # 6250
P = 128
NW1 = 49                      # L1 windows (own nodes), node l = p*49 + k
NPAD1 = P * NW1               # 6272
NW2 = 392                     # L2 windows (all nodes), node n = p2*392 + w2
NPAD2 = P * NW2               # 50176
XSPLIT = 32768                # x table split for int16 gather indices
L1_WCHUNK = 4                 # L1 windows per gather chunk
L2_WCHUNK = 8                # L2 windows per gather chunk
PTK = 8                       # L1 p-table write batch (windows)
PSK = 8                       # L2 partial write batch (windows)
PADVAL = 600.0                # drel pad: never matches iota (0..511)

bf16 = ml_dtypes.bfloat16


def _wrap_call(flat_idx):
    """int16 wrapped layout for one gather call: slot i -> [i%16, i//16]."""
    n = len(flat_idx)
    w = flat_idx.astype(np.int16).reshape(n // 16, 16).T.copy()
    return np.tile(w, (8, 1))  # [128, n/16]


def _build_layer(per_core, nwin, wchunk, nbuckets):
    """Dense-rank schedule shared by both layers.

    per_core[c][b] = (idx, win, slot) arrays for core c, bucket b.
    Returns (idx_arrays[c], drel_arrays[c], chunks meta, nranks).
    chunks: list of dicts with
      calls:   [(bucket, nidx, cum16)]
      ranks:   [(rid, msg_lr, q0, span)]          # S-build info, rank order
      windows: [(w, [(rid, blk)...])]             # matmul ops per window
    """
    # pre-sort each core/bucket by window (stable)
    srt = []
    for c in range(NCORES):
        row = []
        for b in range(nbuckets):
            idx, win, slot = per_core[c][b]
            o = np.argsort(win, kind="stable")
            row.append((idx[o], win[o], slot[o]))
        srt.append(row)

    chunks = []
    idx_cols = [[] for _ in range(NCORES)]
    drel_cols = [[] for _ in range(NCORES)]
    rank_id = 0
    cum16 = 0
    for w0 in range(0, nwin, wchunk):
        w1 = min(w0 + wchunk, nwin)
        calls = []
        ranks_meta = []
        # per-window matmul op lists
        win_ops = {w: [] for w in range(w0, w1)}
        msg_off = 0
        for b in range(nbuckets):
            sel = []
            cnts = []
            for c in range(NCORES):
                idx, win, slot = srt[c][b]
                lo = np.searchsorted(win, w0, "left")
                hi = np.searchsorted(win, w1, "left")
                sel.append((idx[lo:hi], win[lo:hi], slot[lo:hi]))
                cnts.append(hi - lo)
            nr = (max(cnts) + P - 1) // P
            if nr == 0:
                continue
            nidx = nr * P
            # per-core slot fill + per-rank window coverage
            cover = np.full((nr, 2), -1, np.int64)  # union [wmin, wmax]
            for c in range(NCORES):
                idx, win, slot = sel[c]
                flat = np.zeros(nidx, np.int64)
                drl = np.full(nidx, PADVAL, np.float64)
                ne = len(idx)
                flat[:ne] = idx
                drl[:ne] = slot + 128 * (win & 3)
                idx_cols[c].append(_wrap_call(flat))
                drel_cols[c].append(drl.reshape(nr, P).T.astype(np.float32))
                # coverage per rank for this core
                for r in range(nr):
                    a, z = r * P, min((r + 1) * P, ne)
                    if a >= ne:
                        break
                    wmin, wmax = win[a], win[z - 1]
                    if cover[r, 0] < 0:
                        cover[r] = (wmin, wmax)
                    else:
                        cover[r, 0] = min(cover[r, 0], wmin)
                        cover[r, 1] = max(cover[r, 1], wmax)
            calls.append((b, nidx, cum16))
            cum16 += nidx // 16
            for r in range(nr):
                wmin, wmax = cover[r]
                if wmin < 0:
                    continue  # fully-padded rank (no core has edges): skip
                span = int(wmax - wmin + 1)
                assert span <= 4, f"rank spans {span} windows"
                rid = rank_id + r
                ranks_meta.append((rid, msg_off + r, int(wmin) & 3, span))
                for w in range(int(wmin), int(wmax) + 1):
                    win_ops[w].append((rid, w - int(wmin)))
            rank_id += nr
            msg_off += nr
        chunks.append({
            "calls": calls,
            "nranks": msg_off,
            "ranks": ranks_meta,
            "windows": [(w, win_ops[w]) for w in range(w0, w1)],
        })
    idx_arr = [np.concatenate(idx_cols[c], axis=1) if idx_cols[c]
               else np.zeros((P, 0), np.int16) for c in range(NCORES)]
    drel_arr = [np.concatenate(drel_cols[c], axis=1) if drel_cols[c]
                else np.zeros((P, 0), np.float32) for c in range(NCORES)]
    return idx_arr, drel_arr, chunks, rank_id


def _build_schedule(edge_index):
    src = np.asarray(edge_index[0], dtype=np.int64)
    dst = np.asarray(edge_index[1], dtype=np.int64)
    deg = np.bincount(dst, minlength=N).astype(np.float32)
    recip = 1.0 / np.maximum(deg, 1.0)

    # L1: dst-partitioned; window/slot from local node l = p*49 + k
    l1 = []
    for c in range(NCORES):
        m = (dst >= c * NLOC) & (dst < (c + 1) * NLOC)
        s, d = src[m], dst[m] - c * NLOC
        win = d % NW1
        slot = d // NW1
        blo = s < XSPLIT
        l1.append([
            (s[blo], win[blo], slot[blo]),
            (s[~blo] - XSPLIT, win[~blo], slot[~blo]),
        ])
    i1, d1, chunks1, R1 = _build_layer(l1, NW1, L1_WCHUNK, 2)

    # L2: src-partitioned; padded global node n = p2*392 + w2
    l2 = []
    for c in range(NCORES):
        m = (src >= c * NLOC) & (src < (c + 1) * NLOC)
        s, d = src[m] - c * NLOC, dst[m]
        n = NPAD1 * (d // NLOC) + (d % NLOC)
        win = n % NW2
        slot = n // NW2
        l2.append([(s, win, slot)])
    i2, d2, chunks2, R2 = _build_layer(l2, NW2, L2_WCHUNK, 1)

    return i1, d1, chunks1, R1, i2, d2, chunks2, R2, recip


def kernel(x, edge_index, w1_l, b1, w1_r, w2_l, b2, w2_r):
    import concourse.bacc as bacc
    import concourse.mybir as mybir
    import concourse.tile as tile
    from concourse.bass_utils import run_bass_kernel_spmd
    from concourse.library_config import mlp
    from concourse.masks import make_identity

    x = np.asarray(x, np.float32)
    i1, d1, chunks1, R1, i2, d2, chunks2, R2, recip = _build_schedule(
        np.asarray(edge_index))
    CR1 = max(ch["nranks"] for ch in chunks1)
    CR2 = max(ch["nranks"] for ch in chunks2)
    CRMAX = max(CR1, CR2)

    xlo = x[:XSPLIT].astype(bf16)
    xhi = x[XSPLIT:].astype(bf16)
    iota_np = np.tile((np.arange(1024) % 512).astype(np.float32)[None, :],
                      (P, 1)).astype(bf16)

    T16_1 = i1[0].shape[1]
    T16_2 = i2[0].shape[1]

    nc = bacc.Bacc("TRN2", dynamic_dma_scratch_size=49152)
    dt = mybir.dt
    t_xlo = nc.declare_dram_parameter("xlo", [XSPLIT, DIN], dt.bfloat16, isOutput=False)
    t_xhi = nc.declare_dram_parameter("xhi", [N - XSPLIT, DIN], dt.bfloat16, isOutput=False)
    t_i1 = nc.declare_dram_parameter("i1", [P, T16_1], dt.int16, isOutput=False)
    t_d1 = nc.declare_dram_parameter("d1", [P, R1], dt.float32, isOutput=False)
    t_i2 = nc.declare_dram_parameter("i2", [P, T16_2], dt.int16, isOutput=False)
    t_d2 = nc.declare_dram_parameter("d2", [P, R2], dt.float32, isOutput=False)
    t_xoT = nc.declare_dram_parameter("xoT", [DIN, NW1, P], dt.bfloat16, isOutput=False)
    t_w1l = nc.declare_dram_parameter("w1l", [DIN, HID], dt.bfloat16, isOutput=False)
    t_w1r = nc.declare_dram_parameter("w1r", [DIN, HID], dt.bfloat16, isOutput=False)
    t_w2l = nc.declare_dram_parameter("w2l", [P, HID // P, OUT], dt.bfloat16, isOutput=False)
    t_w2r = nc.declare_dram_parameter("w2r", [P, HID // P, OUT], dt.bfloat16, isOutput=False)
    t_b1 = nc.declare_dram_parameter("b1", [P, HID // P], dt.float32, isOutput=False)
    t_b2 = nc.declare_dram_parameter("b2r", [P, OUT], dt.float32, isOutput=False)
    t_rc = nc.declare_dram_parameter("rc", [P, NW1], dt.float32, isOutput=False)
    t_iota = nc.declare_dram_parameter("iota", [P, 1024], dt.bfloat16, isOutput=False)
    t_out = nc.declare_dram_parameter("out", [P, NW1, OUT], dt.float32, isOutput=True)

    t_p = nc.dram_tensor("ptab", [P, NW1, DIN], dt.bfloat16)        # row l = p*49+k
    t_partial = nc.dram_tensor("partial", [P, NW2, OUT], dt.bfloat16)
    t_rs = nc.dram_tensor("rsout", [P // NCORES, NW2, OUT], dt.bfloat16)

    AluOp = mybir.AluOpType
    AF = mybir.ActivationFunctionType

    with tile.TileContext(nc) as tc:
        with tc.tile_pool(name="const", bufs=1) as cpool, \
             tc.tile_pool(name="msg", bufs=3) as mpool, \
             tc.tile_pool(name="sm", bufs=16) as spool, \
             tc.tile_pool(name="work", bufs=3) as wpool, \
             tc.tile_pool(name="stage", bufs=2) as stpool, \
             tc.tile_pool(name="epi", bufs=1) as epool, \
             tc.tile_pool(name="psumA", bufs=2, space="PSUM") as ppool, \
             tc.tile_pool(name="psumB", bufs=2, space="PSUM") as ppoolb, \
             tc.tile_pool(name="psumC", bufs=1, space="PSUM") as ppoolc:
            nc.gpsimd.load_library(mlp)
            ident = cpool.tile([P, P], dt.bfloat16)
            make_identity(nc, ident[:])
            i1_t = cpool.tile([P, T16_1], dt.int16)
            nc.sync.dma_start(i1_t[:], t_i1[:])
            i2_t = cpool.tile([P, T16_2], dt.int16)
            nc.sync.dma_start(i2_t[:], t_i2[:])
            iota_t = cpool.tile([P, 1024], dt.bfloat16)
            nc.sync.dma_start(iota_t[:], t_iota[:])
            d1_t = cpool.tile([P, R1], dt.float32)
            nc.sync.dma_start(d1_t[:], t_d1[:])
            d2_t = cpool.tile([P, R2], dt.float32)
            nc.sync.dma_start(d2_t[:], t_d2[:])
            xoT_t = cpool.tile([DIN, NW1, P], dt.bfloat16)
            nc.sync.dma_start(xoT_t[:], t_xoT[:])
            w1l_t = cpool.tile([DIN, HID], dt.bfloat16)
            nc.sync.dma_start(w1l_t[:], t_w1l[:])
            w1r_t = cpool.tile([DIN, HID], dt.bfloat16)
            nc.sync.dma_start(w1r_t[:], t_w1r[:])
            w2l_t = cpool.tile([P, HID // P, OUT], dt.bfloat16)
            nc.sync.dma_start(w2l_t[:], t_w2l[:])
            w2r_t = cpool.tile([P, HID // P, OUT], dt.bfloat16)
            nc.sync.dma_start(w2r_t[:], t_w2r[:])
            b1_t = cpool.tile([P, HID // P], dt.float32)
            nc.sync.dma_start(b1_t[:], t_b1[:])
            b2_t = cpool.tile([P, OUT], dt.float32)
            nc.sync.dma_start(b2_t[:], t_b2[:])
            rc_t = cpool.tile([P, NW1], dt.float32)
            nc.sync.dma_start(rc_t[:], t_rc[:])
            qbuf = cpool.tile([P, NW1, OUT], dt.float32)
            ptA = cpool.tile([P, PTK, DIN], dt.bfloat16)
            ptB = cpool.tile([P, PTK, DIN], dt.bfloat16)
            nc.vector.memset(ptA[:], 0.0)
            nc.vector.memset(ptB[:], 0.0)

            # ---------------- Layer 1 ----------------
            pt_tiles = [ptA, ptB]
            pt_pend = 0          # windows staged in current pt tile
            pt_w0 = 0
            pt_i = 0
            for ch in chunks1:
                cr = ch["nranks"]
                if cr == 0:
                    continue
                msg = mpool.tile([P, cr, DIN], dt.bfloat16, tag="msg")
                off = 0
                for b, nidx, cum16 in ch["calls"]:
                    tbl = t_xlo[:] if b == 0 else t_xhi[:]
                    nc.gpsimd.dma_gather(
                        msg[:, off:off + nidx // P, :], tbl,
                        i1_t[:, cum16:cum16 + nidx // 16],
                        nidx, nidx, DIN, single_packet=False)
                    off += nidx // P
                rank_info = {rid: (lr, q0, span) for rid, lr, q0, span in ch["ranks"]}
                S_tiles = {}
                for w, ops in ch["windows"]:
                    for rid, blk in ops:
                        if rid not in S_tiles:
                            lr, q0, span = rank_info[rid]
                            S = spool.tile([P, 512], dt.bfloat16, tag="S1")
                            nc.vector.tensor_scalar(
                                out=S[:, :span * P],
                                in0=iota_t[:, q0 * P:(q0 + span) * P],
                                scalar1=d1_t[:, rid:rid + 1], scalar2=None,
                                op0=AluOp.is_equal)
                            S_tiles[rid] = S
                    pagg = ppool.tile([P, P], dt.float32, tag="pagg")
                    if not ops:
                        nc.vector.memset(pagg[:], 0.0)
                    for j, (rid, blk) in enumerate(ops):
                        lr = rank_info[rid][0]
                        nc.tensor.matmul(
                            pagg[:], lhsT=S_tiles[rid][:, blk * P:(blk + 1) * P],
                            rhs=msg[:, lr, :],
                            start=(j == 0), stop=(j == len(ops) - 1))
                    am = wpool.tile([P, P], dt.bfloat16, tag="am")
                    nc.scalar.activation(am[:], pagg[:], AF.Copy,
                                         scale=rc_t[:, w:w + 1])
                    pamT = ppoolc.tile([P, P], dt.bfloat16, tag="pamT")
                    nc.tensor.transpose(out=pamT[:], in_=am[:], identity=ident[:])
                    amT = wpool.tile([P, P], dt.bfloat16, tag="amT")
                    nc.scalar.activation(amT[:], pamT[:], AF.Copy)
                    pq = ppoolc.tile([P, OUT], dt.float32, tag="pq")
                    qq = ppoolc.tile([P, OUT], dt.float32, tag="qq")
                    for bjj in range(HID // P):
                        ph = ppoolb.tile([P, P], dt.float32, tag="ph")
                        nc.tensor.matmul(ph[:], lhsT=w1l_t[:, bjj * P:(bjj + 1) * P],
                                         rhs=amT[:], start=True, stop=False)
                        nc.tensor.matmul(ph[:], lhsT=w1r_t[:, bjj * P:(bjj + 1) * P],
                                         rhs=xoT_t[:, w, :], start=False, stop=True)
                        hT = wpool.tile([P, P], dt.bfloat16, tag="hT")
                        nc.scalar.activation(hT[:], ph[:], AF.Relu,
                                             bias=b1_t[:, bjj:bjj + 1])
                        nc.tensor.matmul(pq[:], lhsT=hT[:], rhs=w2l_t[:, bjj, :],
                                         start=(bjj == 0), stop=(bjj == 3))
                        nc.tensor.matmul(qq[:], lhsT=hT[:], rhs=w2r_t[:, bjj, :],
                                         start=(bjj == 0), stop=(bjj == 3))
                    nc.scalar.activation(qbuf[:, w, :], qq[:], AF.Copy)
                    pt = pt_tiles[pt_i]
                    nc.scalar.activation(pt[:, pt_pend, :OUT], pq[:], AF.Copy)
                    pt_pend += 1
                    if pt_pend == PTK or w == NW1 - 1:
                        nc.sync.dma_start(t_p[:, pt_w0:pt_w0 + pt_pend, :],
                                          pt[:, :pt_pend, :])
                        pt_w0 += pt_pend
                        pt_pend = 0
                        pt_i ^= 1

            # ---------------- Layer 2 ----------------
            t_p_flat = t_p[:].rearrange("p k j -> (p k) j")
            ps_pend = 0
            ps_w0 = 0
            for ch in chunks2:
                cr = ch["nranks"]
                if cr == 0:
                    continue
                msg = mpool.tile([P, cr, DIN], dt.bfloat16, tag="msg")
                for b, nidx, cum16 in ch["calls"]:
                    nc.gpsimd.dma_gather(
                        msg[:, :nidx // P, :], t_p_flat,
                        i2_t[:, cum16:cum16 + nidx // 16],
                        nidx, nidx, DIN, single_packet=False)
                rank_info = {rid: (lr, q0, span) for rid, lr, q0, span in ch["ranks"]}
                S_tiles = {}
                for w, ops in ch["windows"]:
                    for rid, blk in ops:
                        if rid not in S_tiles:
                            lr, q0, span = rank_info[rid]
                            S = spool.tile([P, 512], dt.bfloat16, tag="S2")
                            nc.vector.tensor_scalar(
                                out=S[:, :span * P],
                                in0=iota_t[:, q0 * P:(q0 + span) * P],
                                scalar1=d2_t[:, rid:rid + 1], scalar2=None,
                                op0=AluOp.is_equal)
                            S_tiles[rid] = S
                    pagg = ppool.tile([P, OUT], dt.float32, tag="pagg")
                    if not ops:
                        nc.vector.memset(pagg[:], 0.0)
                    for j, (rid, blk) in enumerate(ops):
                        lr = rank_info[rid][0]
                        nc.tensor.matmul(
                            pagg[:], lhsT=S_tiles[rid][:, blk * P:(blk + 1) * P],
                            rhs=msg[:, lr, :OUT],
                            start=(j == 0), stop=(j == len(ops) - 1))
                    if ps_pend == 0:
                        ps = stpool.tile([P, PSK, OUT], dt.bfloat16, tag="ps")
                    nc.scalar.activation(ps[:, ps_pend, :], pagg[:], AF.Copy)
                    ps_pend += 1
                    if ps_pend == PSK or w == NW2 - 1:
                        nc.sync.dma_start(t_partial[:, ps_w0:ps_w0 + ps_pend, :],
                                          ps[:, :ps_pend, :])
                        ps_w0 += ps_pend
                        ps_pend = 0

            # ---------------- ReduceScatter + epilogue ----------------
            nc.gpsimd.collective_compute(
                "ReduceScatter", AluOp.add, replica_groups=[list(range(NCORES))],
                ins=[t_partial[:]], outs=[t_rs[:]])

            rsb = epool.tile([P, NW1, OUT], dt.bfloat16, tag="rsb")
            nc.sync.dma_start(
                rsb[:], t_rs[:].rearrange("a (b k) j -> (a b) k j", b=NCORES))
            zt = epool.tile([P, NW1, OUT], dt.float32, tag="zt")
            nc.vector.tensor_tensor(
                out=zt[:], in0=rsb[:],
                in1=rc_t[:, :, None].to_broadcast([P, NW1, OUT]), op=AluOp.mult)
            nc.vector.tensor_tensor(out=zt[:], in0=zt[:], in1=qbuf[:], op=AluOp.add)
            nc.vector.tensor_tensor(
                out=zt[:], in0=zt[:],
                in1=b2_t[:, None, :].to_broadcast([P, NW1, OUT]), op=AluOp.add)
            mneg = epool.tile([P, NW1, 1], dt.float32, tag="mneg")
            nc.vector.tensor_reduce(mneg[:], zt[:], axis=mybir.AxisListType.X,
                                    op=AluOp.max, negate=True)
            nc.vector.tensor_tensor(
                out=zt[:], in0=zt[:],
                in1=mneg[:].to_broadcast([P, NW1, OUT]), op=AluOp.add)
            ez = epool.tile([P, NW1, OUT], dt.float32, tag="ez")
            nc.scalar.activation(ez[:], zt[:], AF.Exp)
            ssum = epool.tile([P, NW1, 1], dt.float32, tag="ssum")
            nc.vector.tensor_reduce(ssum[:], ez[:], axis=mybir.AxisListType.X,
                                    op=AluOp.add)
            lsum = epool.tile([P, NW1, 1], dt.float32, tag="lsum")
            nc.scalar.activation(lsum[:], ssum[:], AF.Ln)
            ot = epool.tile([P, NW1, OUT], dt.float32, tag="ot")
            nc.vector.tensor_tensor(
                out=ot[:], in0=zt[:],
                in1=lsum[:].to_broadcast([P, NW1, OUT]), op=AluOp.subtract)
            nc.sync.dma_start(t_out[:], ot[:])

    nc.compile()

    b2b_np = np.tile(np.asarray(b2, np.float32)[None, :], (P, 1))
    in_maps = []
    for c in range(NCORES):
        xl = np.zeros((NPAD1, DIN), np.float32)
        xl[:NLOC] = x[c * NLOC:(c + 1) * NLOC]
        xoT = np.ascontiguousarray(
            xl.reshape(P, NW1, DIN).transpose(2, 1, 0)).astype(bf16)
        rcf = np.ones(NPAD1, np.float32)
        rcf[:NLOC] = recip[c * NLOC:(c + 1) * NLOC]
        rcc = rcf.reshape(P, NW1).copy()
        in_maps.append({
            "xlo": xlo, "xhi": xhi,
            "i1": i1[c], "d1": d1[c], "i2": i2[c], "d2": d2[c],
            "xoT": xoT, "rc": rcc, "iota": iota_np,
            "w1l": np.asarray(w1_l).astype(bf16),
            "w1r": np.asarray(w1_r).astype(bf16),
            "w2l": np.ascontiguousarray(
                np.asarray(w2_l).astype(bf16).reshape(HID // P, P, OUT)
                .transpose(1, 0, 2)),
            "w2r": np.ascontiguousarray(
                np.asarray(w2_r).astype(bf16).reshape(HID // P, P, OUT)
                .transpose(1, 0, 2)),
            "b1": np.asarray(b1, np.float32).reshape(HID // P, P).T.copy(),
            "b2r": b2b_np,
        })
    res = run_bass_kernel_spmd(nc, in_maps, list(range(NCORES)))
    out = np.concatenate(
        [res.results[c]["out"].reshape(NPAD1, OUT)[:NLOC] for c in range(NCORES)],
        axis=0)
    kernel.last_results = res
    kernel.last_nc = nc
    return out.astype(np.float32)
